# revision 1
# baseline (speedup 1.0000x reference)
"""MoE GPT forward on 8 Trainium2 NeuronCores — v3.

Sharding: token-parallel residual stream (256 tokens/core), feature-major
attention (scoresT layout, PE ones-matmul softmax sums), expert-parallel MoE
(1 expert/core/layer, routing replicated from the dispatch allgather),
token-sharded vocab head (full head_w streamed per core, no collective).

Wire format: bf16 for kv exchange, MoE dispatch/combine payloads; f32 gate
logits ride the dispatch AG bitcast into bf16 columns. Weights bf16 on PE
(psum f32), host-relaid into p-major blobs so each weight tensor is one DMA.
Residual x, LN, softmax normalization, routing in f32.
"""
import sys
sys.path.insert(0, '/opt/trn_rl_repo')
from contextlib import ExitStack
import numpy as np

V, S, H, NH, L, E, B = 32000, 1024, 768, 12, 2, 8, 2
DH = H // NH            # 64
FF = 4 * H              # 3072
T = B * S               # 2048
CAP = T // E            # 256
NCORE = 8
TL = T // NCORE         # 256 local tokens per core
HJ = H // 128           # 6
KT = S // 128           # 8 key tiles per batch
MFF = FF // 128         # 24
HE = H + 16             # dispatch payload width in bf16 cols (768 + 8 f32 lg)
NV = 500                # head vocab chunk
NB = V // NV            # 64
GRP4 = [[0, 1, 2, 3], [4, 5, 6, 7]]
GRP8 = [[0, 1, 2, 3, 4, 5, 6, 7]]

# params blob column offsets (f32)
P_LN1S, P_LN1B, P_LN2S, P_LN2B = 0, H, 2 * H, 3 * H
P_VB, P_OUTB, P_B2 = 4 * H, 5 * H, 6 * H
P_B1, P_KB, P_QB, P_GW = 7 * H, 7 * H + MFF, 7 * H + MFF + HJ, 7 * H + MFF + 2 * HJ
PB = P_GW + HJ * E

_BUILT = {}


def _build(debug=False):
    import concourse.bass as bass
    import concourse.mybir as mybir
    import concourse.tile as tile
    from concourse import bacc
    from concourse.bass import ts, ds
    from concourse.masks import make_identity

    f32 = mybir.dt.float32
    f32r = mybir.dt.float32r
    bf16 = mybir.dt.bfloat16
    i32 = mybir.dt.int32
    AF = mybir.ActivationFunctionType
    OP = mybir.AluOpType
    AX = mybir.AxisListType

    nc = bacc.Bacc("TRN2", target_bir_lowering=False, debug=False,
                   num_devices=NCORE)

    def din(name, shape, dt=f32):
        return nc.dram_tensor(name, shape, dt, kind="ExternalInput").ap()

    emb_l = din("emb_l", [TL, H])
    pos_l = din("pos_l", [TL, H])
    tpos_l = din("tpos_l", [TL, 1], i32)
    slotoff = din("slotoff", [CAP, 1], i32)
    myexp = din("myexp", [2, 1], i32)
    akqv = [din(f"akqv_{l}", [128, 3, HJ, H], f32r) for l in range(L)]
    wo_w = [din(f"wo_{l}", [128, HJ, H], f32r) for l in range(L)]
    MDT = [None] * L  # per-layer MoE dtypes, filled below
    w1_w = [din(f"w1_{l}", [128, MFF, H], f32r if l == 0 else bf16)
            for l in range(L)]
    w2_w = [din(f"w2_{l}", [128, MFF, H], f32r if l == 0 else bf16)
            for l in range(L)]
    prm_w = [din(f"prm_{l}", [128, PB]) for l in range(L)]
    fin_w = din("fin_w", [128, 2 * H])
    hw_w = din("hw_w", [128, NB, HJ, NV], bf16)

    out_l = nc.dram_tensor("out_l", [TL, V], bf16, kind="ExternalOutput").ap()
    dbg = {}
    if debug:
        def dout(name, shape):
            dbg[name] = nc.dram_tensor("dbg_" + name, shape, f32,
                                       kind="ExternalOutput").ap()
        dout('xe', [TL, H])
        for l in range(L):
            dout(f'xa{l}', [TL, H])
            dout(f'x{l}', [TL, H])
            dout(f'lg{l}', [T, E])
            dout(f'rt{l}', [4, T])
            dout(f'h2{l}', [CAP, H])

    with tile.TileContext(nc) as tc, ExitStack() as top:
        dram = top.enter_context(tc.tile_pool(name="dram", bufs=1, space="DRAM"))
        const = top.enter_context(tc.tile_pool(name="const", bufs=1))
        persist = top.enter_context(tc.tile_pool(name="persist", bufs=1))
        sb = top.enter_context(tc.tile_pool(name="sb", bufs=1))

        def dtile(name, shape, dt=f32, shared=False):
            return dram.tile(shape, dt, tag=name, name=name,
                             addr_space="Shared" if shared else "Local")

        HTL = H * TL
        kv_in = [dtile(f"kv_in{l}", [2 * HTL], f32r) for l in range(L)]
        kv_out = [dtile(f"kv_out{l}", [4 * 2 * HTL], f32r) for l in range(L)]
        HEL = [H + E, HE]                  # payload cols per layer
        H2DT = [f32, bf16]
        DLDT = [f32, bf16]
        h2l_in = [dtile(f"h2l_in{l}", [TL, HEL[l]], H2DT[l]) for l in range(L)]
        h2l_out = [dtile(f"h2l_out{l}", [T, HEL[l]], H2DT[l], True)
                   for l in range(L)]
        dl_in = [dtile(f"dl_in{l}", [CAP, H], DLDT[l]) for l in range(L)]
        dl_out = [dtile(f"dl_out{l}", [T, H], DLDT[l], True) for l in range(L)]
        scr_srcgs = [dtile(f"scr_srcgs{l}", [T, 2]) for l in range(L)]
        scr_slot = [dtile(f"scr_slot{l}", [T, 1]) for l in range(L)]
        scr_idx = dtile("scr_idx", [T])
        scr_ml = [dtile(f"scr_ml{l}", [16, T]) for l in range(L)]

        # ---- constants ----
        ident = const.tile([128, 128], f32)
        make_identity(nc, ident)
        identb = const.tile([128, 128], bf16)
        nc.vector.tensor_copy(identb[:], ident[:])
        onesf = const.tile([128, 1], f32)
        nc.vector.memset(onesf[:], 1.0)
        ones16 = const.tile([16, 1], f32)
        nc.vector.memset(ones16[:], 1.0)
        ones1x16f = const.tile([1, 16], f32)
        nc.vector.memset(ones1x16f[:], 1.0)
        ones1x16 = const.tile([1, 16], f32r)
        nc.vector.tensor_copy(ones1x16[:], ones1x16f[:])
        iota16 = const.tile([16, 1], i32)
        nc.gpsimd.iota(iota16[:], [[0, 1]], channel_multiplier=1)
        iota16f = const.tile([16, 1], f32)
        nc.vector.tensor_copy(iota16f[:], iota16[:])
        iota8 = const.tile([128, 8], i32)
        nc.gpsimd.iota(iota8[:], [[1, 8]], channel_multiplier=0)
        iota8f = const.tile([128, 8], f32)
        nc.vector.tensor_copy(iota8f[:], iota8[:])
        zeros16 = const.tile([16, T], f32)
        nc.vector.memset(zeros16[:], 0.0)
        eps_t = const.tile([128, 1], f32)
        nc.vector.memset(eps_t[:], 1e-5)
        tokid_i = const.tile([128, 16], i32)
        nc.gpsimd.iota(tokid_i[:], [[128, 16]], channel_multiplier=1)
        tokid_f = const.tile([128, 16], f32)
        nc.vector.tensor_copy(tokid_f[:], tokid_i[:])
        ones64b = const.tile([128, 64], f32r)
        nc.vector.tensor_copy(ones64b[:], onesf[:].to_broadcast([128, 64]))
        ones_1x128 = const.tile([1, 128], f32r)
        nc.vector.tensor_copy(ones_1x128[:], ones1x16f[:, :1].to_broadcast([1, 128]))
        iota_t = const.tile([128, T], f32)
        with tc.tile_pool(name="tmpc", bufs=1) as tmpc:
            iota_t_i = tmpc.tile([128, T], i32)
            nc.gpsimd.iota(iota_t_i[:], [[1, T]], channel_multiplier=0)
            nc.vector.tensor_copy(iota_t[:], iota_t_i[:])
        iota_p_i = const.tile([128, 1], i32)
        nc.gpsimd.iota(iota_p_i[:], [[0, 1]], channel_multiplier=1)
        slot_base = const.tile([128, 2], f32)
        nc.vector.tensor_copy(slot_base[:, 0:1], iota_p_i[:])
        nc.vector.tensor_scalar_add(slot_base[:, 1:2], slot_base[:, 0:1], 129.0)
        nc.vector.tensor_scalar_add(slot_base[:, 0:1], slot_base[:, 0:1], 1.0)

        x_sb = persist.tile([128, 2, H], f32, tag="x_sb")

        # ================= embedding =================
        for k in range(2):
            emb = sb.tile([128, H], f32, tag="emb", bufs=2)
            nc.sync.dma_start(emb[:], emb_l[ds(128 * k, 128), :])
            post = sb.tile([128, H], f32, tag="post", bufs=2)
            nc.sync.dma_start(post[:], pos_l[ds(128 * k, 128), :])
            nc.vector.tensor_add(x_sb[:, k, :], emb[:], post[:])
        if debug:
            nc.sync.dma_start(dbg['xe'].rearrange("(k p) d -> p k d", p=128), x_sb[:])

        def layer_norm(dst, src_view, s_ap, b_ap):
            """One-pass LN over [128, 2, H]; s_ap/b_ap are [128, H] APs."""
            s_bc = s_ap[:, None, :].to_broadcast([128, 2, H])
            b_bc = b_ap[:, None, :].to_broadcast([128, 2, H])
            mean = sb.tile([128, 2, 1], f32, tag="ln_m", bufs=2)
            nc.vector.tensor_reduce(mean[:], src_view[:], axis=AX.X, op=OP.add)
            nc.vector.tensor_scalar_mul(mean[:], mean[:], 1.0 / H)
            xm = sb.tile([128, 2, H], f32, tag="ln_xm", bufs=1)
            nc.vector.tensor_tensor(xm[:], src_view[:],
                                    mean[:].to_broadcast([128, 2, H]),
                                    op=OP.subtract)
            sq = sb.tile([128, 2, H], f32, tag="ln_sq", bufs=1)
            nc.vector.tensor_tensor(sq[:], xm[:], xm[:], op=OP.mult)
            var = sb.tile([128, 2, 1], f32, tag="ln_v", bufs=2)
            nc.vector.tensor_reduce(var[:], sq[:], axis=AX.X, op=OP.add)
            nc.vector.tensor_scalar_mul(var[:], var[:], 1.0 / H)
            sd = sb.tile([128, 2, 1], f32, tag="ln_sd", bufs=2)
            nc.scalar.activation(sd[:], var[:], AF.Sqrt, bias=eps_t[:, :1])
            rstd = sb.tile([128, 2, 1], f32, tag="ln_r", bufs=2)
            nc.vector.reciprocal(rstd[:], sd[:])
            nc.vector.tensor_tensor(dst[:], xm[:],
                                    rstd[:].to_broadcast([128, 2, H]),
                                    op=OP.mult)
            nc.vector.tensor_tensor(dst[:], dst[:], s_bc, op=OP.mult)
            nc.vector.tensor_tensor(dst[:], dst[:], b_bc, op=OP.add)

        def transpose_2H(src_view, dst):
            """src [128,2,H] f32 token-major -> dst [128, HJ, TL] (any dtype)."""
            with tc.tile_pool(name="pst", bufs=3, space="PSUM") as pst:
                for j in range(HJ):
                    pt = pst.tile([128, 2, 128], f32, tag="pt", bufs=3)
                    for k in range(2):
                        nc.tensor.transpose(pt[:, k, :], src_view[:, k, ts(j, 128)],
                                            ident[:])
                    nc.vector.tensor_copy(
                        dst[:, j, :].rearrange("p (k c) -> p k c", k=2), pt[:])

        # ================= layers =================
        for l in range(L):
            with ExitStack() as lyr:
                lprm = lyr.enter_context(tc.tile_pool(name="lprm", bufs=1))
                prms = lprm.tile([128, PB], f32, tag="prms")
                nc.sync.dma_start(prms[:], prm_w[l])
                abuf_cm = tc.tile_pool(name="abuf", bufs=1)
                abuf = abuf_cm.__enter__()
                h1 = abuf.tile([128, 2, H], f32, tag="h1")
                layer_norm(h1, x_sb[:], prms[:, ds(P_LN1S, H)],
                           prms[:, ds(P_LN1B, H)])
                h1T = abuf.tile([128, HJ, TL], f32r, tag="h1T")
                transpose_2H(h1, h1T)

                # ---- K, V then allgather; Q during allgather ----
                kqT = abuf.tile([128, 2, HJ, TL], f32r, tag="kqT")
                v_sb = abuf.tile([128, 2, H], f32r, tag="v_sb")
                with tc.tile_pool(name="psq", bufs=2, space="PSUM") as psq, \
                     tc.tile_pool(name="wkv", bufs=1) as wkv:
                    awk = wkv.tile([128, 2, HJ, H], f32r, tag="awk")
                    nc.sync.dma_start(awk[:, 0, :, :], akqv[l][:, 0, :, :])
                    nc.sync.dma_start(awk[:, 1, :, :], akqv[l][:, 2, :, :])
                    for m in range(HJ):
                        pq = psq.tile([128, TL], f32, tag="pq", bufs=2)
                        for j in range(HJ):
                            nc.tensor.matmul(
                                pq[:], awk[:, 0, j, ts(m, 128)], h1T[:, j, :],
                                start=(j == 0), stop=(j == HJ - 1))
                        nc.vector.tensor_scalar_add(
                            kqT[:, 0, m, :], pq[:],
                            prms[:, P_KB + m:P_KB + m + 1])
                    vb_bc = prms[:, ds(P_VB, H)]
                    for k in range(2):
                        for nn in range(2):
                            pv = psq.tile([128, 384], f32, tag="pv", bufs=2)
                            for j in range(HJ):
                                nc.tensor.matmul(
                                    pv[:], h1T[:, j, ts(k, 128)],
                                    awk[:, 1, j, ds(384 * nn, 384)],
                                    start=(j == 0), stop=(j == HJ - 1))
                            nc.vector.tensor_add(
                                v_sb[:, k, ds(384 * nn, 384)], pv[:],
                                vb_bc[:, ds(384 * nn, 384)])
                    # stage + allgather K,V
                    kT_view = kv_in[l][:HTL].rearrange(
                        "(p m t) -> p m t", p=128, t=TL)
                    v_view = kv_in[l][HTL:].rearrange(
                        "(p k d) -> p k d", p=128, d=H)
                    nc.scalar.dma_start(kT_view, kqT[:, 0, :, :])
                    nc.scalar.dma_start(v_view, v_sb[:])
                    nc.gpsimd.collective_compute(
                        "AllGather", OP.bypass, replica_groups=GRP4,
                        ins=[kv_in[l][:]], outs=[kv_out[l][:]])

                # ---- attention (Q computed during the allgather) ----
                ctxP = abuf.tile([128, HJ, TL], f32r, tag="ctxP")
                kvo = kv_out[l][:].rearrange("(r s) -> r s", s=2 * HTL)
                with tc.tile_pool(name="psa", bufs=4, space="PSUM") as psa, \
                     tc.tile_pool(name="ab2", bufs=2) as ab2, \
                     tc.tile_pool(name="abk", bufs=1) as abk:
                    awq = abk.tile([128, HJ, H], f32r, tag="awq")
                    nc.sync.dma_start(awq[:], akqv[l][:, 1, :, :])
                    for m in range(HJ):
                        pq2 = psa.tile([128, TL], f32, tag="pq2", bufs=2)
                        for j in range(HJ):
                            nc.tensor.matmul(
                                pq2[:], awq[:, j, ts(m, 128)], h1T[:, j, :],
                                start=(j == 0), stop=(j == HJ - 1))
                        nc.vector.tensor_scalar_add(
                            kqT[:, 1, m, :], pq2[:],
                            prms[:, P_QB + m:P_QB + m + 1])
                    kall = abk.tile([128, 4, HJ, TL], f32r, tag="kall")
                    nc.sync.dma_start(
                        kall[:], kvo[:, :HTL].rearrange("r (p m t) -> p r m t",
                                                        p=128, t=TL))
                    kTh2 = [abk.tile([128, S], f32r, tag=f"kTh{par}",
                                     name=f"kTh{par}", bufs=1)
                            for par in range(2)]
                    zrow = ab2.tile([64, S], f32, tag="zrow", bufs=1)
                    nc.vector.memset(zrow[:], 0.0)
                    nc.vector.tensor_copy(kTh2[0][ds(64, 64), :], zrow[:])
                    nc.vector.tensor_copy(kTh2[1][ds(0, 64), :], zrow[:])
                    vsrc = kvo[:, HTL:].rearrange("r (p k d) -> p r k d",
                                                  p=128, d=H)

                    def attn_finish(h, expT, vh):
                        po = 64 * (h % 2)
                        jq = h // 2
                        psum_s = psa.tile([64, TL], f32, tag="psum_s", bufs=2)
                        for kk in range(KT):
                            nc.tensor.matmul(psum_s[:], ones64b[:], expT[:, kk, :],
                                             start=(kk == 0), stop=(kk == KT - 1))
                        rbc = ab2.tile([64, TL], f32, tag="rbc", bufs=2)
                        nc.vector.reciprocal(rbc[:], psum_s[:])
                        pc = psa.tile([64, TL], f32, tag="pc", bufs=2)
                        for kk in range(KT):
                            nc.tensor.matmul(pc[:], vh[:, kk, :], expT[:, kk, :],
                                             start=(kk == 0), stop=(kk == KT - 1))
                        nc.vector.tensor_tensor(ctxP[ds(po, 64), jq, :], pc[:],
                                                rbc[:], op=OP.mult)

                    pend = None
                    for h in range(NH):
                        po = 64 * (h % 2)
                        jq = h // 2
                        kTh = kTh2[h % 2]
                        nc.vector.tensor_copy(
                            kTh[ds(po, 64), :].rearrange("p (r t) -> p r t", r=4),
                            kall[ds(po, 64), :, jq, :])
                        vh = ab2.tile([128, KT, 64], f32r, tag="vh", bufs=2)
                        for r in range(4):
                            nc.sync.dma_start(
                                vh[:, ds(2 * r, 2), :],
                                vsrc[:, r, :, ds(64 * h, 64)])
                        expT = ab2.tile([128, KT, TL], f32r, tag="expT", bufs=2)
                        for kp in range(KT // 2):
                            pss = psa.tile([128, 2, TL], f32, tag="pss", bufs=2)
                            for i in range(2):
                                nc.tensor.matmul(pss[:, i, :],
                                                 kTh[:, ts(2 * kp + i, 128)],
                                                 kqT[:, 1, jq, :],
                                                 start=True, stop=True)
                            nc.scalar.activation(
                                expT[:, ds(2 * kp, 2), :], pss[:], AF.Exp,
                                scale=1.0 / np.sqrt(DH))
                        # softmax-denominator + ctx of the PREVIOUS head: PE
                        # issues them after this head's scores, so it never
                        # stalls on the Activation engine's exp.
                        if pend is not None:
                            attn_finish(*pend)
                        pend = (h, expT, vh)
                    attn_finish(*pend)

                # ---- out-proj + residual ----
                with tc.tile_pool(name="pso", bufs=2, space="PSUM") as pso, \
                     tc.tile_pool(name="wop", bufs=1) as wop:
                    wo_sb = wop.tile([128, HJ, H], f32r, tag="wo_sb")
                    nc.sync.dma_start(wo_sb[:], wo_w[l])
                    ob_bc = prms[:, ds(P_OUTB, H)]
                    for k in range(2):
                        for nn in range(2):
                            pol = pso.tile([128, 384], f32, tag="pol", bufs=2)
                            for m in range(HJ):
                                nc.tensor.matmul(pol[:], ctxP[:, m, ts(k, 128)],
                                                 wo_sb[:, m, ds(384 * nn, 384)],
                                                 start=(m == 0), stop=(m == HJ - 1))
                            sl = ds(384 * nn, 384)
                            nc.vector.tensor_add(x_sb[:, k, sl], x_sb[:, k, sl],
                                                 pol[:])
                        nc.vector.tensor_add(x_sb[:, k, :], x_sb[:, k, :],
                                             ob_bc[:, :])
                if debug:
                    nc.sync.dma_start(
                        dbg[f'xa{l}'].rearrange("(k p) d -> p k d", p=128), x_sb[:])
                abuf_cm.__exit__(None, None, None)

                # ---- LN2 + gate logits + dispatch allgather ----
                mbuf = lyr.enter_context(tc.tile_pool(name="mbuf", bufs=1))
                h2 = mbuf.tile([128, 2, H], f32, tag="h2")
                layer_norm(h2, x_sb[:], prms[:, ds(P_LN2S, H)],
                           prms[:, ds(P_LN2B, H)])
                h2T = mbuf.tile([128, HJ, TL], f32, tag="h2T")
                transpose_2H(h2, h2T)
                if l == 0:
                    h2b = h2
                else:
                    h2b = mbuf.tile([128, 2, H], bf16, tag="h2b")
                    nc.scalar.copy(h2b[:], h2[:])
                lg_loc = sb.tile([128, 2, E], f32, tag="lg_loc")
                with tc.tile_pool(name="psg", bufs=2, space="PSUM") as psg:
                    for k in range(2):
                        pg = psg.tile([128, E], f32, tag="pg", bufs=2)
                        for j in range(HJ):
                            nc.tensor.matmul(
                                pg[:], h2T[:, j, ts(k, 128)],
                                prms[:, ds(P_GW + E * j, E)],
                                start=(j == 0), stop=(j == HJ - 1))
                        nc.vector.tensor_copy(lg_loc[:, k, :], pg[:])
                nc.scalar.dma_start(
                    h2l_in[l][:, :H].rearrange("(k p) d -> p k d", p=128), h2b[:])
                nc.scalar.dma_start(
                    h2l_in[l][:, H:].rearrange("(k p) e -> p k e", p=128),
                    lg_loc[:] if l == 0 else lg_loc[:].bitcast(bf16))
                nc.gpsimd.collective_compute(
                    "AllGather", OP.bypass, replica_groups=GRP8,
                    ins=[h2l_in[l][:]], outs=[h2l_out[l][:]])


                # ---- routing (replicated on all cores) ----
                with tc.tile_pool(name="rt", bufs=1) as rt, \
                     tc.tile_pool(name="psr", bufs=2, space="PSUM") as psr:
                    lg = rt.tile([128, 16, E], f32, tag="lg")
                    lgsrc = (h2l_out[l][:, H:] if l == 0
                             else h2l_out[l][:, H:].bitcast(f32))
                    nc.sync.dma_start(
                        lg[:], lgsrc.rearrange("(c p) e -> p c e", p=128))
                    if debug:
                        nc.sync.dma_start(
                            dbg[f'lg{l}'].rearrange("(c p) e -> p c e", p=128),
                            lg[:])
                    ex = rt.tile([128, 16, E], f32, tag="ex")
                    nc.scalar.activation(ex[:], lg[:], AF.Exp)
                    mx = rt.tile([128, 16, 1], f32, tag="mx")
                    nc.vector.tensor_reduce(mx[:], ex[:], axis=AX.X, op=OP.max)
                    sm = rt.tile([128, 16, 1], f32, tag="sm")
                    nc.vector.tensor_reduce(sm[:], ex[:], axis=AX.X, op=OP.add)
                    rsm = rt.tile([128, 16, 1], f32, tag="rsm")
                    nc.vector.reciprocal(rsm[:], sm[:])
                    gp = rt.tile([128, 16], f32, tag="gp")
                    nc.vector.tensor_tensor(gp[:], mx[:, :, 0], rsm[:, :, 0],
                                            op=OP.mult)
                    eq = rt.tile([128, 16, E], f32, tag="eq")
                    nc.vector.tensor_tensor(eq[:], ex[:],
                                            mx[:].to_broadcast([128, 16, E]),
                                            op=OP.is_equal)
                    eqi = rt.tile([128, 16, E], f32, tag="eqi")
                    nc.vector.tensor_tensor(
                        eqi[:], eq[:],
                        iota8f[:, None, :].to_broadcast([128, 16, E]), op=OP.mult)
                    idxf = rt.tile([128, 16, 1], f32, tag="idxf")
                    nc.vector.tensor_reduce(idxf[:], eqi[:], axis=AX.X, op=OP.add)
                    # token-order idx vector -> [1, T] -> PE-broadcast to 16 rows
                    nc.sync.dma_start(
                        scr_idx[:].rearrange("(c p) -> p c", p=128),
                        idxf[:, :, 0])
                    idx1 = rt.tile([1, T], f32r, tag="idx1")
                    nc.sync.dma_start(idx1[:], scr_idx[None, :].bitcast(f32r))
                    idxb = rt.tile([16, T], f32, tag="rt16", bufs=5, name="idxb")
                    for q in range(4):
                        pb = psr.tile([16, 512], f32, tag="pb", bufs=2)
                        nc.tensor.matmul(pb[:], ones1x16[:], idx1[:, ts(q, 512)],
                                         start=True, stop=True)
                        nc.vector.tensor_copy(idxb[:, ts(q, 512)], pb[:])
                    maskT = rt.tile([16, T], f32, tag="rt16", bufs=5, name="maskT")
                    nc.vector.tensor_scalar(maskT[:], idxb[:], iota16f[:, :1],
                                            None, op0=OP.is_equal)
                    locs = rt.tile([16, T], f32, tag="rt16", bufs=5, name="locs")
                    nc.vector.tensor_tensor_scan(locs[:], maskT[:], zeros16[:],
                                                 0.0, op0=OP.add, op1=OP.add)
                    elig = rt.tile([16, T], f32, tag="rt16", bufs=5, name="elig")
                    nc.vector.tensor_scalar(elig[:], locs[:], float(CAP), None,
                                            op0=OP.is_le)
                    nc.vector.tensor_tensor(elig[:], elig[:], maskT[:], op=OP.mult)
                    ml = rt.tile([16, T], f32, tag="rt16", bufs=5, name="ml")
                    nc.vector.tensor_tensor(ml[:], elig[:], locs[:], op=OP.mult)
                    ml_tm = rt.tile([128, 16], f32, tag="ml_tm")
                    pml = psr.tile([128, 16], f32, tag="pml", bufs=2)
                    for c in range(16):
                        nc.tensor.matmul(pml[:, c:c + 1], ml[:, ts(c, 128)],
                                         ones16[:], start=True, stop=True)
                    nc.vector.tensor_copy(ml_tm[:], pml[:])
                    kept_tm = rt.tile([128, 16], f32, tag="kept_tm")
                    nc.vector.tensor_scalar(kept_tm[:], ml_tm[:], 0.5, None,
                                            op0=OP.is_ge)
                    a_tm = rt.tile([128, 16], f32, tag="a_tm")
                    nc.vector.scalar_tensor_tensor(
                        out=a_tm[:], in0=idxf[:, :, 0], scalar=float(CAP),
                        in1=ml_tm[:], op0=OP.mult, op1=OP.add)
                    nc.vector.tensor_scalar_add(a_tm[:], a_tm[:], -1.0)
                    src_tm = rt.tile([128, 16], f32, tag="src_tm")
                    nc.vector.tensor_tensor(src_tm[:], a_tm[:], kept_tm[:],
                                            op=OP.mult)
                    gs_tm = rt.tile([128, 16], f32, tag="gs_tm")
                    nc.vector.tensor_tensor(gs_tm[:], gp[:], kept_tm[:], op=OP.mult)
                    nc.scalar.dma_start(
                        scr_srcgs[l][:, 0].rearrange("(c p) -> p c", p=128),
                        src_tm[:])
                    nc.scalar.dma_start(
                        scr_srcgs[l][:, 1].rearrange("(c p) -> p c", p=128),
                        gs_tm[:])
                    zsl = rt.tile([128, 16], f32, tag="zsl")
                    nc.vector.memset(zsl[:], 0.0)
                    nc.scalar.dma_start(
                        scr_slot[l][:, 0].rearrange("(c p) -> p c", p=128), zsl[:])
                    ssrc = rt.tile([128, 16], f32, tag="ssrc")
                    nc.vector.tensor_scalar(ssrc[:], kept_tm[:], -1e6, 1e6,
                                            op0=OP.mult, op1=OP.add)
                    nc.vector.tensor_add(ssrc[:], ssrc[:], src_tm[:])
                    ssrc_i = rt.tile([128, 16], i32, tag="ssrc_i")
                    nc.vector.tensor_copy(ssrc_i[:], ssrc[:])
                    for c in range(16):
                        nc.gpsimd.indirect_dma_start(
                            out=scr_slot[l][:], in_=tokid_f[:, c:c + 1],
                            in_offset=None,
                            out_offset=bass.IndirectOffsetOnAxis(
                                ap=ssrc_i[:, c:c + 1], axis=0),
                            bounds_check=T - 1, oob_is_err=False)
                    if debug:
                        nc.sync.dma_start(
                            dbg[f'rt{l}'][0, :].rearrange("(c p) -> p c", p=128),
                            idxf[:, :, 0])
                        nc.sync.dma_start(
                            dbg[f'rt{l}'][1, :].rearrange("(c p) -> p c", p=128),
                            src_tm[:])
                        nc.sync.dma_start(
                            dbg[f'rt{l}'][2, :].rearrange("(c p) -> p c", p=128),
                            gs_tm[:])
                        nc.sync.dma_start(
                            dbg[f'rt{l}'][3, :].rearrange("(c p) -> p c", p=128),
                            kept_tm[:])

                # ---- MoE FFN (this core's expert) ----
                mdt = f32r if l == 0 else bf16
                xsT = mbuf.tile([128, HJ, CAP], mdt, tag="xsT")
                with tc.tile_pool(name="psx", bufs=4, space="PSUM") as psx, \
                     tc.tile_pool(name="mb", bufs=2) as mb:
                    xgs = []
                    for k in range(2):
                        soff = mb.tile([128, 1], i32, tag="soff", bufs=2)
                        nc.sync.dma_start(soff[:], slotoff[ds(128 * k, 128), :])
                        offf = mb.tile([128, 1], f32, tag="offf", bufs=2)
                        nc.gpsimd.indirect_dma_start(
                            out=offf[:], out_offset=None, in_=scr_slot[l][:],
                            in_offset=bass.IndirectOffsetOnAxis(ap=soff[:, :1],
                                                                axis=0))
                        offi = mb.tile([128, 1], i32, tag="offi", bufs=2)
                        nc.vector.tensor_copy(offi[:], offf[:])
                        xg = mb.tile([128, HEL[l]], H2DT[l], tag="xg", bufs=2)
                        nc.gpsimd.indirect_dma_start(
                            out=xg[:], out_offset=None, in_=h2l_out[l][:],
                            in_offset=bass.IndirectOffsetOnAxis(ap=offi[:, :1],
                                                                axis=0))
                        xgs.append(xg)
                    for j in range(HJ):
                        pxt = psx.tile([128, 2, 128], H2DT[l], tag="xtp", bufs=4)
                        for k in range(2):
                            nc.tensor.transpose(pxt[:, k, :], xgs[k][:, ts(j, 128)],
                                                ident[:] if l == 0 else identb[:])
                        nc.vector.tensor_copy(
                            xsT[:, j, :].rearrange("p (k c) -> p k c", k=2),
                            pxt[:])
                h1T_m = mbuf.tile([128, MFF, CAP], mdt, tag="h1T_m")
                MC = 2 if l == 0 else 4  # m-chunks per streamed weight load
                with tc.tile_pool(name="psm", bufs=2, space="PSUM") as psm, \
                     tc.tile_pool(name="wst", bufs=3) as wst:
                    for mq in range(MFF // MC):
                        w1c = wst.tile([128, MC, H], mdt, tag="w1c", bufs=3)
                        nc.sync.dma_start(w1c[:], w1_w[l][:, ds(MC * mq, MC), :])
                        for mi in range(MC):
                            m = MC * mq + mi
                            ph = psm.tile([128, CAP], f32, tag="ph", bufs=2)
                            for j in range(HJ):
                                nc.tensor.matmul(ph[:], w1c[:, mi, ts(j, 128)],
                                                 xsT[:, j, :],
                                                 start=(j == 0), stop=(j == HJ - 1))
                            nc.scalar.activation(
                                h1T_m[:, m, :], ph[:], AF.Gelu,
                                bias=prms[:, P_B1 + m:P_B1 + m + 1])
                dsb = sb.tile([128, 2, H], DLDT[l], tag="dsb", name=f"dsb{l}")
                with tc.tile_pool(name="psd", bufs=1, space="PSUM") as psd, \
                     tc.tile_pool(name="wst2", bufs=3) as wst2:
                    b2_bc = prms[:, ds(P_B2, H)]
                    pdt = [[psd.tile([128, 384], f32, tag=f"pd{k}{nn}",
                                     name=f"pd{k}{nn}_{l}", bufs=1)
                            for nn in range(2)] for k in range(2)]
                    for mq in range(MFF // MC):
                        w2c = wst2.tile([128, MC, H], mdt, tag="w2c", bufs=3)
                        nc.sync.dma_start(w2c[:], w2_w[l][:, ds(MC * mq, MC), :])
                        for mi in range(MC):
                            m = MC * mq + mi
                            for k in range(2):
                                for nn in range(2):
                                    nc.tensor.matmul(pdt[k][nn][:],
                                                     h1T_m[:, m, ts(k, 128)],
                                                     w2c[:, mi, ds(384 * nn, 384)],
                                                     start=(m == 0),
                                                     stop=(m == MFF - 1))
                    for k in range(2):
                        for nn in range(2):
                            sl = ds(384 * nn, 384)
                            nc.vector.tensor_add(dsb[:, k, sl], pdt[k][nn][:],
                                                 b2_bc[:, sl])
                if debug:
                    dsbf = sb.tile([128, 2, H], f32, tag="dsbf")
                    nc.vector.tensor_copy(dsbf[:], dsb[:])
                    nc.sync.dma_start(
                        dbg[f'h2{l}'].rearrange("(k p) d -> p k d", p=128), dsbf[:])
                nc.scalar.dma_start(
                    dl_in[l][:].rearrange("(k p) d -> p k d", p=128), dsb[:])
                nc.gpsimd.collective_compute(
                    "AllGather", OP.bypass, replica_groups=GRP8,
                    ins=[dl_in[l][:]], outs=[dl_out[l][:]])

                # ---- combine ----
                for k in range(2):
                    tp = sb.tile([128, 1], i32, tag="tp", bufs=2)
                    nc.sync.dma_start(tp[:], tpos_l[ds(128 * k, 128), :])
                    sgf = sb.tile([128, 2], f32, tag="sgf", bufs=2)
                    nc.gpsimd.indirect_dma_start(
                        out=sgf[:], out_offset=None, in_=scr_srcgs[l][:],
                        in_offset=bass.IndirectOffsetOnAxis(ap=tp[:, :1], axis=0))
                    srci = sb.tile([128, 1], i32, tag="srci", bufs=2)
                    nc.vector.tensor_copy(srci[:], sgf[:, 0:1])
                    dg = sb.tile([128, H], DLDT[l], tag="dg", name=f"dg{l}",
                                 bufs=2)
                    nc.gpsimd.indirect_dma_start(
                        out=dg[:], out_offset=None, in_=dl_out[l][:],
                        in_offset=bass.IndirectOffsetOnAxis(ap=srci[:, :1], axis=0))
                    if l == 0:
                        dgf = dg
                    else:
                        dgf = sb.tile([128, H], f32, tag="dgf", bufs=2)
                        nc.vector.tensor_copy(dgf[:], dg[:])
                    nc.vector.scalar_tensor_tensor(
                        out=x_sb[:, k, :], in0=dgf[:], scalar=sgf[:, 1:2],
                        in1=x_sb[:, k, :], op0=OP.mult, op1=OP.add)
                if debug:
                    nc.sync.dma_start(
                        dbg[f'x{l}'].rearrange("(k p) d -> p k d", p=128), x_sb[:])

        # ================= final LN + head (no collective) =================
        with ExitStack() as fin:
            fb = fin.enter_context(tc.tile_pool(name="fb", bufs=1))
            fparam = fb.tile([128, 2 * H], f32, tag="fparam")
            nc.sync.dma_start(fparam[:], fin_w)
            hf = fb.tile([128, 2, H], f32, tag="hf")
            layer_norm(hf, x_sb[:], fparam[:, ds(0, H)], fparam[:, ds(H, H)])
            hfT = fb.tile([128, HJ, TL], bf16, tag="hfT")
            transpose_2H(hf, hfT)
            psh = fin.enter_context(tc.tile_pool(name="psh", bufs=2, space="PSUM"))
            hwp = fin.enter_context(tc.tile_pool(name="hwp", bufs=3))
            for g in range(NB // 4):
                osb = [hwp.tile([128, 4, NV], bf16, tag=f"osb{t_}",
                                name=f"osb{t_}", bufs=2) for t_ in range(2)]
                for i in range(4):
                    n = 4 * g + i
                    rhs_n = hwp.tile([128, HJ, NV], bf16, tag="rhs_n", bufs=3)
                    nc.sync.dma_start(rhs_n[:], hw_w[:, n, :, :])
                    for t_ in range(2):
                        po_ = psh.tile([128, NV], f32, tag="po_", bufs=2)
                        for j in range(HJ):
                            nc.tensor.matmul(po_[:], hfT[:, j, ts(t_, 128)],
                                             rhs_n[:, j, :],
                                             start=(j == 0), stop=(j == HJ - 1))
                        if t_ == 0:
                            nc.vector.tensor_copy(osb[t_][:, i, :], po_[:])
                        else:
                            nc.scalar.copy(osb[t_][:, i, :], po_[:])
                for t_ in range(2):
                    nc.scalar.dma_start(
                        out_l[ds(128 * t_, 128), ds(4 * NV * g, 4 * NV)],
                        osb[t_][:].rearrange("p i v -> p (i v)"))

    nc.compile()
    return nc


def _shard_inputs(inputs):
    f = lambda a: np.ascontiguousarray(np.asarray(a), dtype=np.float32)
    try:
        from ml_dtypes import bfloat16 as bf
    except ImportError:
        import jax.numpy as jnp
        bf = jnp.bfloat16
    h = lambda a: np.ascontiguousarray(np.asarray(a, dtype=np.float32).astype(bf))
    ids = np.asarray(inputs['input_ids']).astype(np.int64).reshape(T)
    tokemb = f(inputs['token_emb'])
    pos = f(inputs['pos_emb'])
    hwT = f(inputs['head_w']).T                                  # [H, V]
    hw_l = h(hwT.reshape(HJ, 128, NB, NV).transpose(1, 2, 0, 3))  # [128,NB,HJ,NV]
    fin_blob = np.empty((128, 2 * H), np.float32)
    fin_blob[:, :H] = np.tile(f(inputs['lnf_scale']).reshape(1, H), (128, 1))
    fin_blob[:, H:] = np.tile(f(inputs['lnf_bias']).reshape(1, H), (128, 1))

    akqv_l, wo_l, w1_l, w2_l, prm_l = [], [], [], [], []
    for l in range(L):
        in_w = f(inputs['attn_in_w'][l])
        in_b = f(inputs['attn_in_b'][l])
        qT = in_w[:H].T.reshape(HJ, 128, HJ, 128).transpose(1, 0, 2, 3)
        kT = in_w[H:2 * H].T.reshape(HJ, 128, HJ, 128).transpose(1, 0, 2, 3)
        vT = in_w[2 * H:].T.reshape(HJ, 128, H).transpose(1, 0, 2)
        A = np.empty((128, 3, HJ, H), np.float32)
        A[:, 0] = kT.reshape(128, HJ, H)
        A[:, 1] = qT.reshape(128, HJ, H)
        A[:, 2] = vT
        akqv_l.append(A)
        wo_l.append(np.ascontiguousarray(f(inputs['attn_out_w'][l]).T.reshape(
            HJ, 128, H).transpose(1, 0, 2)))
        P = np.zeros((128, PB), np.float32)
        bc = lambda vv: np.tile(f(vv).reshape(1, H), (128, 1))
        P[:, P_LN1S:P_LN1S + H] = bc(inputs['ln1_scale'][l])
        P[:, P_LN1B:P_LN1B + H] = bc(inputs['ln1_bias'][l])
        P[:, P_LN2S:P_LN2S + H] = bc(inputs['ln2_scale'][l])
        P[:, P_LN2B:P_LN2B + H] = bc(inputs['ln2_bias'][l])
        P[:, P_VB:P_VB + H] = np.tile(in_b[2 * H:].reshape(1, H), (128, 1))
        P[:, P_OUTB:P_OUTB + H] = bc(inputs['attn_out_b'][l])
        P[:, P_KB:P_KB + HJ] = in_b[H:2 * H].reshape(HJ, 128).T
        P[:, P_QB:P_QB + HJ] = in_b[:H].reshape(HJ, 128).T
        P[:, P_GW:P_GW + HJ * E] = f(inputs['gate_w'][l]).T.reshape(
            HJ, 128, E).transpose(1, 0, 2).reshape(128, HJ * E)
        prm_l.append(P)

    in_maps = []
    for c in range(NCORE):
        sl = slice(TL * c, TL * (c + 1))
        m = {
            'emb_l': np.ascontiguousarray(tokemb[ids[sl]]),
            'pos_l': np.ascontiguousarray(pos[np.arange(TL * c, TL * (c + 1)) % S]),
            'tpos_l': np.arange(TL * c, TL * (c + 1), dtype=np.int32).reshape(TL, 1),
            'slotoff': np.arange(CAP * c, CAP * (c + 1),
                                 dtype=np.int32).reshape(CAP, 1),
            'myexp': np.array([[c], [c]], dtype=np.int32),
            'fin_w': fin_blob,
            'hw_w': hw_l,
        }
        for l in range(L):
            m[f'akqv_{l}'] = akqv_l[l]
            m[f'wo_{l}'] = wo_l[l]
            cst = (lambda a: np.ascontiguousarray(a)) if l == 0 else h
            m[f'w1_{l}'] = cst(
                f(inputs['w1'][l, c]).reshape(HJ, 128, MFF, 128).transpose(
                    1, 2, 0, 3).reshape(128, MFF, H))
            m[f'w2_{l}'] = cst(
                f(inputs['w2'][l, c]).reshape(MFF, 128, H).transpose(1, 0, 2))
            P = prm_l[l].copy()
            P[:, P_B2:P_B2 + H] = np.tile(
                f(inputs['b2'][l, c]).reshape(1, H), (128, 1))
            P[:, P_B1:P_B1 + MFF] = f(inputs['b1'][l, c]).reshape(MFF, 128).T
            m[f'prm_{l}'] = P
        in_maps.append(m)
    return in_maps


def run(inputs, debug=False, trace=False):
    from concourse.bass_utils import run_bass_kernel_spmd
    key = bool(debug)
    if key not in _BUILT:
        _BUILT[key] = _build(debug=debug)
    nc = _BUILT[key]
    in_maps = _shard_inputs(inputs)
    return run_bass_kernel_spmd(nc, in_maps, core_ids=list(range(NCORE)),
                                trace=trace)


def kernel(**inputs):
    res = run(inputs, debug=False)
    out = np.concatenate(
        [np.asarray(res.results[c]['out_l']).astype(np.float32)
         for c in range(NCORE)], axis=0)
    return out.reshape(B, S, V)

